# revision 77
# baseline (speedup 1.0000x reference)
"""Trainium2 Bass kernel for CellFoundation self-attention (B=4, S=1024, HID=1024, H=16, DH=64).

Sharding: 8 cores as 4 (batch) x 2 (head-group) grid. Each core handles one
batch and 8 heads (512 of the 1024 QKV output dims).

Design (cost-model driven, ACT-paced):
  - All projections run as fp8 e4m3 DoubleRow matmuls with residual splits:
    w ~ w8 + dw8, x ~ x8 + dx8. "3p" chunks compute w8x8 + dw8x8 + w8dx8
    (error ~0.2%); "2p" chunks compute (w8+dw8)x8 in one DR pass, leaving
    only the x-quantization error (~half the fp8-score-trick error).
  - RoPE fused on DVE + PE permutation matmul (as before).
  - Scores transposed [k, q] per head pair; pairs may use the fp8 DoubleRow
    broadcast trick (both operands stride-0 across DR slots -> 2x score,
    absorbed into the exp scale).
  - exp on ACT is the critical stream: 64 tiles of [128,1024] ~= 66us. The
    whole schedule keeps ACT busy; PE work (scores, PV, projections) dribbles
    into the slack between score matmuls.
  - PV in natural layout: ctx[q, od] with lhsT = E[k, q-chunk] (stationary)
    and rhs = V_hat[k, 65] (64 dims + scaled-ones denominator column). Output
    free size is 65, so PV costs 65 cyc/MM. Normalize is then per-partition:
    one DVE reciprocal over both heads' denominator columns + one
    scalar_tensor_tensor per head folding 1/den and the V bias.
  - Output written natural [S, OD] bf16; host just concatenates.
"""

import sys

if "/opt/trn_rl_repo" not in sys.path:
    sys.path.insert(0, "/opt/trn_rl_repo")

from collections import deque
from contextlib import ExitStack

import ml_dtypes
import numpy as np

import concourse.bass as bass  # noqa: F401
import concourse.tile as tile
from concourse import bacc, mybir
from concourse.bass_utils import run_bass_kernel_spmd

F32 = mybir.dt.float32
BF16 = mybir.dt.bfloat16
F8 = mybir.dt.float8e4
AF = mybir.ActivationFunctionType
MUL = mybir.AluOpType.mult
ADD = mybir.AluOpType.add
DR = mybir.MatmulPerfMode.DoubleRow

B, S, HID, H, DH = 4, 1024, 1024, 16, 64
P = 128
NCORES = 8
HG = 2
HL = H // HG        # 8 local heads
OD = HL * DH        # 512
KC = HID // P       # 8 contraction chunks
CP = KC // 2        # 4 chunk pairs
NT = S // P         # 8 k-tiles
QCH = 512
NQC = S // QCH      # 2
NJ = QCH // P       # 4 q sub-chunks per QCH
NPAIR = HL // 2     # 4
MASK_NEG = -60.0
W_SCALE = 32.0

# Per-pair config: (proj_mode, score_fp8).
#  proj_mode "3p": w8x8 + dw8x8 + w8dx8 (12 MMs, ~0.2% err)
#  proj_mode "2p": (w8+dw8)x8 (8 MMs, x-quant err ~1.6e-2 via scores; pair0
#                  uses this so its chunks never wait on the dx8 DMA)
#  score_fp8: rope outputs quantized e4m3, DR broadcast trick (0.9e-2/pair)
PAIR_CFG = [("3p", False), ("2p", True), ("3p", True), ("3p", True)]
# processing order: cheap fp8 pairs first (startup filler is heavy), the
# bf16-score pair last (no filler left to dribble by then)
PAIR_ORDER = [1, 2, 3, 0]
WARMUP_MMS = 6
TILE_BUDGET_NS = 900.0   # PE dribble budget per exp tile beyond fp8 scores


def _build_nc(debug: bool = False):
    nc = bacc.Bacc("TRN2", target_bir_lowering=False, debug=debug)

    d_x8 = nc.dram_tensor("x8", [P, KC, S], F8, kind="ExternalInput")
    d_dx8 = nc.dram_tensor("dx8", [P, KC, S], F8, kind="ExternalInput")
    d_wq = nc.dram_tensor("wq8", [P, NPAIR, KC, 2, P], F8, kind="ExternalInput")
    d_wk = nc.dram_tensor("wk8", [P, NPAIR, KC, 2, P], F8, kind="ExternalInput")
    d_wv = nc.dram_tensor("wv8", [P, KC, 2, OD], F8, kind="ExternalInput")
    d_cs = nc.dram_tensor("cs", [P, NQC, 2, QCH], BF16, kind="ExternalInput")
    d_R = nc.dram_tensor("rmat", [P, P], BF16, kind="ExternalInput")
    d_cf = nc.dram_tensor("cf", [P, 16], F32, kind="ExternalInput")  # bq|bk|mask
    d_bv = nc.dram_tensor("bv", [1, OD], F32, kind="ExternalInput")
    d_out = nc.dram_tensor("out", [S, OD], BF16, kind="ExternalOutput")

    _uid = [0]

    def _nm(pfx):
        _uid[0] += 1
        return f"{pfx}_{_uid[0]}"

    with tile.TileContext(nc) as tc, ExitStack() as ctx:
        const = ctx.enter_context(tc.tile_pool(name="const", bufs=1))
        qkp = ctx.enter_context(tc.tile_pool(name="qkp", bufs=8))
        rt = ctx.enter_context(tc.tile_pool(name="rt", bufs=14))
        ep = ctx.enter_context(tc.tile_pool(name="ep", bufs=30))
        op = ctx.enter_context(tc.tile_pool(name="op", bufs=3))
        npo = ctx.enter_context(tc.tile_pool(name="npo", bufs=8))
        pp = ctx.enter_context(tc.tile_pool(name="pp", bufs=2, space="PSUM"))
        sp = ctx.enter_context(tc.tile_pool(name="sp", bufs=2, space="PSUM"))
        cv = ctx.enter_context(tc.tile_pool(name="cv", bufs=2, space="PSUM"))

        # ---- SBUF constants / inputs ----
        t_x8 = const.tile([P, KC, S], F8)
        t_dx8 = const.tile([P, KC, S], F8)
        t_wq = const.tile([P, NPAIR, KC, 2, P], F8)
        t_wk = const.tile([P, NPAIR, KC, 2, P], F8)
        t_wv = const.tile([P, KC, 2, OD], F8)
        t_cs = const.tile([P, NQC, 2, QCH], BF16)
        t_R = const.tile([P, P], BF16)
        t_cf = const.tile([P, 16], F32)
        t_bv1 = const.tile([1, OD], F32)
        t_bvb = const.tile([P, OD], F32)
        t_v = [const.tile([P, HL, 65], BF16, tag=f"v{vt}", name=f"vsb{vt}") for vt in range(NT)]

        # small consts on the gpsimd ring (Pool-side DGE), big streams on sync
        nc.gpsimd.dma_start(t_cf[:], d_cf[:])
        nc.gpsimd.dma_start(t_R[:], d_R[:])
        nc.gpsimd.dma_start(t_bv1[:], d_bv[:])
        nc.gpsimd.dma_start(t_cs[:, 1], d_cs[:, 1])
        first = PAIR_ORDER[0]
        nc.sync.dma_start(t_x8[:, :, 0:QCH], d_x8[:, :, 0:QCH])
        nc.sync.dma_start(t_wq[:, first], d_wq[:, first])
        nc.sync.dma_start(t_wk[:, first], d_wk[:, first])
        nc.sync.dma_start(t_cs[:, 0], d_cs[:, 0])
        nc.sync.dma_start(t_dx8[:, :, 0:QCH], d_dx8[:, :, 0:QCH])
        nc.sync.dma_start(t_x8[:, :, QCH:S], d_x8[:, :, QCH:S])
        nc.sync.dma_start(t_wv[:], d_wv[:])
        nc.sync.dma_start(t_dx8[:, :, QCH:S], d_dx8[:, :, QCH:S])
        for hp in PAIR_ORDER[1:]:
            nc.sync.dma_start(t_wq[:, hp], d_wq[:, hp])
            nc.sync.dma_start(t_wk[:, hp], d_wk[:, hp])

        nc.gpsimd.partition_broadcast(t_bvb, t_bv1)

        # exp table warm (no DMA dependency)
        t_z = rt.tile([1, 4], F32, tag="warm", bufs=1)
        nc.vector.memset(t_z[:], 0.0)
        t_wz = rt.tile([1, 4], F32, tag="warm2", bufs=1)
        nc.scalar.activation(t_wz, t_z, AF.Exp)

        # PE HAM/p-state warmup during the input DMA wait
        t_wu = rt.tile([P, QCH], BF16, tag="wu", bufs=1)
        nc.vector.memset(t_wu[:], 0.0)
        p_wu = pp.tile([P, QCH], F32, tag="pp")
        for _ in range(WARMUP_MMS):
            nc.tensor.matmul(p_wu[:], t_wu[:, 0:P], t_wu[:], start=True, stop=True)

        # ---- chunk builders: lists of (est_ns, closure) + finalize ----
        def make_qk_chunk(hp, which, dest_tile, t, rot_on_pe=False, stt_eng=None):
            t_w = t_wq if which == "q" else t_wk
            tsl = slice(t * QCH, (t + 1) * QCH)
            mode = PAIR_CFG[hp][0]
            state = {}

            def get_p():
                if "p" not in state:
                    state["p"] = pp.tile([P, QCH], F32, tag="pp", name=_nm("pq"))
                return state["p"]

            mms = []
            if mode == "3p":
                def mk3(prod, cp):
                    def f():
                        p = get_p()
                        wi = 1 if prod == 1 else 0
                        rhs_t = t_dx8 if prod == 2 else t_x8
                        nc.tensor.matmul(
                            p,
                            t_w[:, hp, 2 * cp : 2 * cp + 2, wi, :],
                            rhs_t[:, 2 * cp : 2 * cp + 2, tsl],
                            start=(prod == 0 and cp == 0),
                            stop=(prod == 2 and cp == CP - 1),
                            perf_mode=DR,
                        )
                    return f
                mms = [(107.0, mk3(prod, cp)) for prod in range(3) for cp in range(CP)]
            else:
                def mk2(c):
                    def f():
                        p = get_p()
                        nc.tensor.matmul(
                            p,
                            t_w[:, hp, c, :, :],
                            t_x8[:, c, tsl].unsqueeze(1).to_broadcast([P, 2, QCH]),
                            start=(c == 0),
                            stop=(c == KC - 1),
                            perf_mode=DR,
                        )
                    return f
                mms = [(107.0, mk2(c)) for c in range(KC)]

            def finalize():
                p = state["p"]
                eng = stt_eng or nc.vector
                scalar = t_cf[:, (0 if which == "q" else 4) + hp : (0 if which == "q" else 4) + hp + 1]
                t_ys = rt.tile([P, QCH], BF16, tag="ys", name=_nm("ys"))
                t_tc = rt.tile([P, QCH], BF16, tag="tc", name=_nm("tc"))
                eng.scalar_tensor_tensor(
                    out=t_ys, in0=p, scalar=scalar, in1=t_cs[:, t, 1], op0=ADD, op1=MUL
                )
                eng.scalar_tensor_tensor(
                    out=t_tc, in0=p, scalar=scalar, in1=t_cs[:, t, 0], op0=ADD, op1=MUL
                )
                if rot_on_pe:
                    # PE permutation matmul: shorter serial DVE chain — used
                    # for the startup-critical pre-stream chunks
                    nc.tensor.matmul(p, t_R[:], t_ys, start=True, stop=True)
                    nc.vector.tensor_add(dest_tile[:], t_tc, p)
                else:
                    # rotate-half as 4 DVE partition-block copies (bf16 SBUF,
                    # 4x mode) -- keeps the PE out of the rope chain; the add
                    # (all-SBUF) runs on the idle GpSimd to decongest DVE
                    t_rot = rt.tile([P, QCH], BF16, tag="rot", name=_nm("rot"))
                    for blk in (0, 64):
                        nc.vector.tensor_copy(t_rot[blk : blk + 32], t_ys[blk + 32 : blk + 64])
                        nc.vector.tensor_copy(t_rot[blk + 32 : blk + 64], t_ys[blk : blk + 32])
                    nc.gpsimd.tensor_add(dest_tile[:], t_tc, t_rot)

            mms.append((0.0, finalize))
            return mms

        def make_v_chunk(vt):
            vtsl = slice(vt * P, (vt + 1) * P)
            state = {}

            def get_p():
                if "p" not in state:
                    state["p"] = pp.tile([P, OD], F32, tag="pp", name=_nm("pv"))
                return state["p"]

            def mkv(prod, cp):
                def f():
                    p = get_p()
                    lhs_t = t_dx8 if prod == 2 else t_x8
                    wi = 1 if prod == 1 else 0
                    nc.tensor.matmul(
                        p,
                        lhs_t[:, 2 * cp : 2 * cp + 2, vtsl],
                        t_wv[:, 2 * cp : 2 * cp + 2, wi, :],
                        start=(prod == 0 and cp == 0),
                        stop=(prod == 2 and cp == CP - 1),
                        perf_mode=DR,
                    )
                return f

            mms = [(107.0, mkv(prod, cp)) for prod in range(3) for cp in range(CP)]

            def finalize():
                p = state["p"]
                nc.vector.tensor_copy(
                    t_v[vt][:, :, 0:64],
                    p[:].rearrange("p (h c) -> p h c", h=HL, c=64),
                )
                # scaled-ones denominator column: V_hat is W_SCALE*V, so the
                # denominator must also be W_SCALE*sum(E) for the ratio to
                # stay correct
                nc.vector.memset(t_v[vt][:, :, 64:65], W_SCALE)

            mms.append((0.0, finalize))
            return mms

        def qk_tiles(hp):
            dt_ = F8 if PAIR_CFG[hp][1] else BF16
            qr = [qkp.tile([P, QCH], dt_, tag=f"qr{t}", name=f"qr{t}_{hp}") for t in range(NQC)]
            kr = [qkp.tile([P, QCH], dt_, tag=f"kr{t}", name=f"kr{t}_{hp}") for t in range(NQC)]
            return qr, kr

        # ---- pre-stream: first pair's q-t0/k-t0 chunks, token-split in
        # 256-col halves for a shorter serial rope chain; q's stts on DVE,
        # k's on the idle GpSimd; rotates land in a borrowed scores-psum
        # tile (sequential groups per bank) ----
        p_first = PAIR_ORDER[0]
        pair_tiles = {p_first: qk_tiles(p_first)}
        qr0, kr0 = pair_tiles[p_first]
        HQ = QCH // 2
        p_preq = pp.tile([P, QCH], F32, tag="pp", name=_nm("ppreq"))
        p_prek = pp.tile([P, QCH], F32, tag="pp", name=_nm("pprek"))
        s_rot = sp.tile([P, 2 * QCH], F32, tag="sp", name=_nm("srot"))

        def pre_mms(which, p_c, half):
            t_w = t_wq if which == "q" else t_wk
            csl = slice(half * HQ, (half + 1) * HQ)
            for c in range(KC):
                nc.tensor.matmul(
                    p_c[:, csl],
                    t_w[:, p_first, c, :, :],
                    t_x8[:, c, csl].unsqueeze(1).to_broadcast([P, 2, HQ]),
                    start=(c == 0), stop=(c == KC - 1), perf_mode=DR,
                )

        def pre_fin(which, p_c, dest, half, rot_off, eng, add_eng=None):
            scalar = t_cf[:, (0 if which == "q" else 4) + p_first : (0 if which == "q" else 4) + p_first + 1]
            csl = slice(half * HQ, (half + 1) * HQ)
            t_ys = rt.tile([P, HQ], BF16, tag="ys", name=_nm("ys"))
            t_tc = rt.tile([P, HQ], BF16, tag="tc", name=_nm("tc"))
            eng.scalar_tensor_tensor(
                out=t_ys, in0=p_c[:, csl], scalar=scalar, in1=t_cs[:, 0, 1, csl], op0=ADD, op1=MUL
            )
            eng.scalar_tensor_tensor(
                out=t_tc, in0=p_c[:, csl], scalar=scalar, in1=t_cs[:, 0, 0, csl], op0=ADD, op1=MUL
            )
            nc.tensor.matmul(
                s_rot[:, rot_off : rot_off + HQ], t_R[:], t_ys, start=True, stop=True
            )
            (add_eng or nc.vector).tensor_add(
                dest[:, csl], t_tc, s_rot[:, rot_off : rot_off + HQ]
            )

        # q: one full-width chunk (3 DVE ops — fewer per-op overheads);
        # k: two halves so kr[:, 0:256] unblocks the first scores early
        for c in range(KC):
            nc.tensor.matmul(
                p_preq,
                t_wq[:, p_first, c, :, :],
                t_x8[:, c, 0:QCH].unsqueeze(1).to_broadcast([P, 2, QCH]),
                start=(c == 0), stop=(c == KC - 1), perf_mode=DR,
            )
        pre_mms("k", p_prek, 0)
        q_scalar = t_cf[:, p_first : p_first + 1]
        t_ysq = rt.tile([P, QCH], BF16, tag="ys", name=_nm("ys"))
        t_tcq = rt.tile([P, QCH], BF16, tag="tc", name=_nm("tc"))
        nc.vector.scalar_tensor_tensor(
            out=t_ysq, in0=p_preq, scalar=q_scalar, in1=t_cs[:, 0, 1], op0=ADD, op1=MUL
        )
        nc.vector.scalar_tensor_tensor(
            out=t_tcq, in0=p_preq, scalar=q_scalar, in1=t_cs[:, 0, 0], op0=ADD, op1=MUL
        )
        nc.tensor.matmul(s_rot[:, 0:QCH], t_R[:], t_ysq, start=True, stop=True)
        nc.vector.tensor_add(qr0[0][:], t_tcq, s_rot[:, 0:QCH])
        pre_mms("k", p_prek, 1)
        pre_fin("k", p_prek, kr0[0], 0, 2 * QCH - 2 * HQ, nc.vector)
        pre_fin("k", p_prek, kr0[0], 1, 2 * QCH - HQ, nc.vector)

        # ---- filler: remaining chunk work, dribbled into PE slack ----
        # items are (est_ns, closure, seg); need(seg) force-pops (in FIFO
        # order) until that segment is fully issued — issue deadlines.
        filler = deque()
        seg_left = {}

        def fpush(seg, items):
            seg_left[seg] = seg_left.get(seg, 0) + len(items)
            for est, f in items:
                filler.append((est, f, seg))

        fpush(f"p{p_first}k1", make_qk_chunk(p_first, "k", kr0[1], 1))
        fpush(f"p{p_first}q1", make_qk_chunk(p_first, "q", qr0[1], 1))
        for hp in PAIR_ORDER[1:]:
            pair_tiles[hp] = qk_tiles(hp)
        pA, pB, pC = PAIR_ORDER[1], PAIR_ORDER[2], PAIR_ORDER[3]

        def qk_seg(hp, which, t):
            qr, kr = pair_tiles[hp]
            dest = (qr if which == "q" else kr)[t]
            fpush(f"p{hp}{which}{t}", make_qk_chunk(hp, which, dest, t))

        qk_seg(pA, "q", 0)
        qk_seg(pA, "k", 0)
        fpush("v0", make_v_chunk(0))
        fpush("v1", make_v_chunk(1))
        qk_seg(pA, "k", 1)
        fpush("v2", make_v_chunk(2))
        fpush("v3", make_v_chunk(3))
        qk_seg(pA, "q", 1)
        fpush("v4", make_v_chunk(4))
        fpush("v5", make_v_chunk(5))
        fpush("v6", make_v_chunk(6))
        fpush("v7", make_v_chunk(7))
        qk_seg(pB, "q", 0)
        qk_seg(pB, "k", 0)
        qk_seg(pB, "k", 1)
        qk_seg(pB, "q", 1)
        # (pair C's chunks follow; all V is issued by here so early units'
        # PV groups unblock and es slots recycle)
        qk_seg(pC, "q", 0)
        qk_seg(pC, "k", 0)
        qk_seg(pC, "k", 1)
        qk_seg(pC, "q", 1)

        vt_issued = [0]  # count of fully-issued V chunks

        def pop_one():
            est, f, seg = filler.popleft()
            f()
            seg_left[seg] -= 1
            if est == 0.0 and seg.startswith("v"):
                vt_issued[0] += 1
            return est

        def pop_filler(budget_ns):
            spent = 0.0
            while filler and spent < budget_ns:
                spent += pop_one()

        def need(seg):
            while seg_left.get(seg, 0) > 0:
                pop_one()

        # pvq: FIFO of (min_vt, est_ns, closure) for PV accumulation groups
        # (one PSUM bank each: 8 MMs + reciprocal + normalize) and out-DMAs
        pvq = deque()

        def pop_pvq(budget_ns):
            spent = 0.0
            while pvq and pvq[0][0] <= vt_issued[0] and spent < budget_ns:
                _, est, f = pvq.popleft()
                f()
                spent += est
            return spent

        d_or = d_out[:].rearrange("(a j p) (hp c) -> p a j hp c", p=P, j=NJ, hp=NPAIR)

        def push_unit_groups(hp_, qc_, es_list, t_out_):
            for j in range(NJ):
                for h in range(2):
                    gi = 2 * j + h
                    last_u = hp_ == PAIR_ORDER[-1] and qc_ == NQC - 1

                    def g(j=j, h=h, gi=gi, es_list=es_list, t_out_=t_out_, hp_=hp_, last_u=last_u):
                        def f():
                            pool_ = pp if (last_u and gi % 2 == 1) else cv
                            p_g = pool_.tile([P, 65], F32, tag="cv" if pool_ is cv else "pp", name=_nm("cv"))
                            for kt in range(NT):
                                nc.tensor.matmul(
                                    p_g,
                                    es_list[kt][:, h * QCH + j * P : h * QCH + (j + 1) * P],
                                    t_v[kt][:, 2 * hp_ + h, :],
                                    start=(kt == 0), stop=(kt == NT - 1),
                                )
                            t_rr = npo.tile([P, 1], F32, tag="rr", name=_nm("rr"))
                            nc.vector.reciprocal(t_rr, p_g[:, 64:65])
                            nc.vector.scalar_tensor_tensor(
                                out=t_out_[:, j, h * DH : (h + 1) * DH],
                                in0=p_g[:, 0:DH],
                                scalar=t_rr,
                                in1=t_bvb[:, (2 * hp_ + h) * DH : (2 * hp_ + h + 1) * DH],
                                op0=MUL, op1=ADD,
                            )
                        return f
                    pvq.append((NT, 250.0, g()))

            last_unit = hp_ == PAIR_ORDER[-1] and qc_ == NQC - 1
            if last_unit:
                # per-j-pair DMAs, re-ordered into the group list so each
                # fires as soon as its half of t_out is normalized
                items = list(pvq)
                for _ in range(8):
                    pvq.pop()
                gs = items[-8:]
                del items
                for j0 in range(0, NJ, 2):
                    def dmaj(qc_=qc_, hp_=hp_, t_out_=t_out_, j0=j0):
                        nc.sync.dma_start(
                            d_or[:, qc_, j0 : j0 + 2, hp_, :], t_out_[:, j0 : j0 + 2]
                        )
                    pvq.extend(gs[4 * (j0 // 2) : 4 * (j0 // 2) + 4])
                    pvq.append((NT, 0.0, dmaj))
            else:
                def dma(qc_=qc_, hp_=hp_, t_out_=t_out_):
                    nc.sync.dma_start(d_or[:, qc_, :, hp_, :], t_out_[:])
                pvq.append((NT, 0.0, dma))

        # ---- the ACT-paced attention stream ----
        def pv_mm(p_g, es_t, j, h, hp_, kt):
            nc.tensor.matmul(
                p_g,
                es_t[:, h * QCH + j * P : h * QCH + (j + 1) * P],
                t_v[kt][:, 2 * hp_ + h, :],
                start=(kt == 0), stop=(kt == NT - 1),
            )

        def norm_group(p_g, j, h, hp_, t_out_, stt_eng):
            t_rr = npo.tile([P, 1], F32, tag="rr", name=_nm("rr"))
            nc.vector.reciprocal(t_rr, p_g[:, 64:65])
            stt_eng.scalar_tensor_tensor(
                out=t_out_[:, j, h * DH : (h + 1) * DH],
                in0=p_g[:, 0:DH],
                scalar=t_rr,
                in1=t_bvb[:, (2 * hp_ + h) * DH : (2 * hp_ + h + 1) * DH],
                op0=MUL, op1=ADD,
            )

        for hp in PAIR_ORDER:
            fp8_pair = PAIR_CFG[hp][1]
            qr_cur, kr_cur = pair_tiles[hp]
            for qc in range(NQC):
                last_unit = hp == PAIR_ORDER[-1] and qc == NQC - 1
                need(f"p{hp}q{qc}")
                if qc == 0:
                    need(f"p{hp}k0")
                t_out = op.tile([P, NJ, 2 * DH], BF16, tag="out")
                streamed = []
                es_tiles = []
                for kt in range(NT):
                    if kt == 4:
                        need(f"p{hp}k1")
                    # scores for this k-tile (2 heads)
                    p_s = sp.tile([P, 2 * QCH], F32, tag="sp")
                    krc = kr_cur[kt // 4]
                    kcol = (kt % 4) * P
                    for half, b0 in ((0, 0), (1, 64)):
                        osl = slice(half * QCH, (half + 1) * QCH)
                        if fp8_pair:
                            nc.tensor.matmul(
                                p_s[:, osl],
                                krc[b0 : b0 + 64, kcol : kcol + P]
                                .unsqueeze(1).to_broadcast([64, 2, P]),
                                qr_cur[qc][b0 : b0 + 64, :]
                                .unsqueeze(1).to_broadcast([64, 2, QCH]),
                                start=True, stop=True, perf_mode=DR,
                            )
                        else:
                            nc.tensor.matmul(
                                p_s[:, osl],
                                krc[b0 : b0 + 64, kcol : kcol + P],
                                qr_cur[qc][b0 : b0 + 64, :],
                                start=True, stop=True,
                            )
                    t_e = ep.tile([P, 2 * QCH], BF16, tag="e")
                    nc.scalar.activation(
                        t_e, p_s, AF.Exp, bias=t_cf[:, 8 + kt : 9 + kt],
                        scale=0.0625 if fp8_pair else 0.125,
                    )
                    es_tiles.append(t_e)

                    s_ns = 213.0 if fp8_pair else 854.0
                    gb = 900.0 if not filler else (500.0 if len(pvq) > 18 else 300.0)
                    g_spent = pop_pvq(gb)
                    fb = TILE_BUDGET_NS + (500.0 if vt_issued[0] >= NT else 0.0)
                    pop_filler(max(0.0, fb - (s_ns - 213.0) - g_spent))

                push_unit_groups(hp, qc, es_tiles, t_out)

        # drain any remaining queued work (last unit's PV + tails)
        while pvq or filler:
            pop_filler(1e9)
            pop_pvq(1e9)

    nc.compile()
    return nc


_NC_CACHE = {}


def _get_nc(debug: bool = False):
    key = bool(debug)
    if key not in _NC_CACHE:
        _NC_CACHE[key] = _build_nc(debug)
    return _NC_CACHE[key]


def _prep_inputs(hidden_states, attention_mask, freqs, Wq, bq, Wk, bk, Wv, bv):
    hidden_states = np.asarray(hidden_states)
    attention_mask = np.asarray(attention_mask)
    freqs = np.asarray(freqs)
    Wq, bq = np.asarray(Wq), np.asarray(bq)
    Wk, bk = np.asarray(Wk), np.asarray(bk)
    Wv, bv = np.asarray(Wv), np.asarray(bv)
    bf = ml_dtypes.bfloat16
    f8 = mybir.dt.np(F8)

    inv = 1.0 / W_SCALE
    cosf = (np.cos(freqs.astype(np.float64)) * inv).astype(np.float32)  # [S, 64]
    sinf = (np.sin(freqs.astype(np.float64)) * inv).astype(np.float32)
    # sign-baked sin for the rotate-half permutation matmul
    sgn = np.ones((64, 1), np.float32)
    sgn[32:] = -1.0
    cos2 = np.tile(cosf.T, (2, 1))                    # [128, S]
    sin2 = np.tile(sinf.T * sgn, (2, 1))              # [128, S], sign-baked
    # cs layout [P, NQC, 2, QCH]
    cs = np.empty((P, NQC, 2, QCH), np.float32)
    for t in range(NQC):
        cs[:, t, 0] = cos2[:, t * QCH : (t + 1) * QCH]
        cs[:, t, 1] = sin2[:, t * QCH : (t + 1) * QCH]
    cs = cs.astype(bf)

    rmat = np.zeros((P, P), np.float32)
    for blk in (0, 64):
        for i in range(32):
            rmat[blk + i + 32, blk + i] = 1.0
            rmat[blk + i, blk + i + 32] = 1.0
    rmat = rmat.astype(bf)

    def split8(a):
        a8 = a.astype(f8)
        d8 = (a - a8.astype(np.float32)).astype(f8)
        return a8, d8

    # x transposed [HID, S] -> [P, KC, S]
    x8s, dx8s = [], []
    for b in range(B):
        xt = np.ascontiguousarray(hidden_states[b].T).astype(np.float32)
        x8, dx8 = split8(xt)
        x8s.append(np.ascontiguousarray(x8.reshape(KC, P, S).transpose(1, 0, 2)))
        dx8s.append(np.ascontiguousarray(dx8.reshape(KC, P, S).transpose(1, 0, 2)))

    masks = []
    for b in range(B):
        m = np.where(attention_mask[b, 0, 0, :] < -1e-5, MASK_NEG, 0.0).astype(np.float32)
        masks.append(np.ascontiguousarray(m.reshape(NT, P).T))

    def pack_w_qk(Wm):
        # [HID, OD] -> [P, NPAIR, KC, 2, P] with (w8, dw8) in axis 3
        w = (Wm * W_SCALE).astype(np.float32)
        w8, dw8 = split8(w)
        out = np.empty((P, NPAIR, KC, 2, P), np.float32)
        for hp in range(NPAIR):
            csl = slice(hp * P, (hp + 1) * P)
            out[:, hp, :, 0, :] = w8.astype(np.float32)[:, csl].reshape(KC, P, P).transpose(1, 0, 2)
            out[:, hp, :, 1, :] = dw8.astype(np.float32)[:, csl].reshape(KC, P, P).transpose(1, 0, 2)
        return np.ascontiguousarray(out.astype(f8))

    def pack_w_v(Wm):
        w = (Wm * W_SCALE).astype(np.float32)
        w8, dw8 = split8(w)
        out = np.empty((P, KC, 2, OD), np.float32)
        out[:, :, 0, :] = w8.astype(np.float32).reshape(KC, P, OD).transpose(1, 0, 2)
        out[:, :, 1, :] = dw8.astype(np.float32).reshape(KC, P, OD).transpose(1, 0, 2)
        return np.ascontiguousarray(out.astype(f8))

    wqs, wks, wvs, cfs, bvs = [], [], [], [], []
    for g in range(HG):
        osl = slice(g * OD, (g + 1) * OD)
        wqs.append(pack_w_qk(Wq[:, osl]))
        wks.append(pack_w_qk(Wk[:, osl]))
        wvs.append(pack_w_v(Wv[:, osl]))
        bqs = (bq[osl] * W_SCALE).reshape(NPAIR, P).T       # [P, 4]
        bks = (bk[osl] * W_SCALE).reshape(NPAIR, P).T
        cfs.append((bqs, bks))
        bvs.append(bv[osl].reshape(1, OD).astype(np.float32))

    in_maps = []
    for c in range(NCORES):
        b, g = c // HG, c % HG
        cf = np.concatenate([cfs[g][0], cfs[g][1], masks[b]], axis=1).astype(np.float32)
        assert cf.shape == (P, 16)
        in_maps.append(
            dict(
                x8=x8s[b], dx8=dx8s[b],
                wq8=wqs[g], wk8=wks[g], wv8=wvs[g],
                cs=cs, rmat=rmat, cf=cf, bv=bvs[g],
            )
        )
    return in_maps


def kernel(hidden_states, attention_mask, freqs, Wq, bq, Wk, bk, Wv, bv, **run_kwargs):
    nc = _get_nc()
    in_maps = _prep_inputs(
        hidden_states, attention_mask, freqs, Wq, bq, Wk, bk, Wv, bv
    )
    res = run_bass_kernel_spmd(nc, in_maps, core_ids=list(range(NCORES)), **run_kwargs)
    out = np.empty((B, S, H * DH), np.float32)
    for c in range(NCORES):
        b, g = c // HG, c % HG
        out[b, :, g * OD : (g + 1) * OD] = np.asarray(res.results[c]["out"]).astype(np.float32)
    if run_kwargs:
        kernel.last_results = res
    return out


# revision 78
# speedup vs baseline: 1.0047x; 1.0047x over previous
"""Trainium2 Bass kernel for CellFoundation self-attention (B=4, S=1024, HID=1024, H=16, DH=64).

Sharding: 8 cores as 4 (batch) x 2 (head-group) grid. Each core handles one
batch and 8 heads (512 of the 1024 QKV output dims).

Design (cost-model driven, ACT-paced):
  - All projections run as fp8 e4m3 DoubleRow matmuls with residual splits:
    w ~ w8 + dw8, x ~ x8 + dx8. "3p" chunks compute w8x8 + dw8x8 + w8dx8
    (error ~0.2%); "2p" chunks compute (w8+dw8)x8 in one DR pass, leaving
    only the x-quantization error (~half the fp8-score-trick error).
  - RoPE fused on DVE + PE permutation matmul (as before).
  - Scores transposed [k, q] per head pair; pairs may use the fp8 DoubleRow
    broadcast trick (both operands stride-0 across DR slots -> 2x score,
    absorbed into the exp scale).
  - exp on ACT is the critical stream: 64 tiles of [128,1024] ~= 66us. The
    whole schedule keeps ACT busy; PE work (scores, PV, projections) dribbles
    into the slack between score matmuls.
  - PV in natural layout: ctx[q, od] with lhsT = E[k, q-chunk] (stationary)
    and rhs = V_hat[k, 65] (64 dims + scaled-ones denominator column). Output
    free size is 65, so PV costs 65 cyc/MM. Normalize is then per-partition:
    one DVE reciprocal over both heads' denominator columns + one
    scalar_tensor_tensor per head folding 1/den and the V bias.
  - Output written natural [S, OD] bf16; host just concatenates.
"""

import sys

if "/opt/trn_rl_repo" not in sys.path:
    sys.path.insert(0, "/opt/trn_rl_repo")

from collections import deque
from contextlib import ExitStack

import ml_dtypes
import numpy as np

import concourse.bass as bass  # noqa: F401
import concourse.tile as tile
from concourse import bacc, mybir
from concourse.bass_utils import run_bass_kernel_spmd

F32 = mybir.dt.float32
BF16 = mybir.dt.bfloat16
F8 = mybir.dt.float8e4
AF = mybir.ActivationFunctionType
MUL = mybir.AluOpType.mult
ADD = mybir.AluOpType.add
DR = mybir.MatmulPerfMode.DoubleRow

B, S, HID, H, DH = 4, 1024, 1024, 16, 64
P = 128
NCORES = 8
HG = 2
HL = H // HG        # 8 local heads
OD = HL * DH        # 512
KC = HID // P       # 8 contraction chunks
CP = KC // 2        # 4 chunk pairs
NT = S // P         # 8 k-tiles
QCH = 512
NQC = S // QCH      # 2
NJ = QCH // P       # 4 q sub-chunks per QCH
NPAIR = HL // 2     # 4
MASK_NEG = -60.0
W_SCALE = 32.0

# Per-pair config: (proj_mode, score_fp8).
#  proj_mode "3p": w8x8 + dw8x8 + w8dx8 (12 MMs, ~0.2% err)
#  proj_mode "2p": (w8+dw8)x8 (8 MMs, x-quant err ~1.6e-2 via scores; pair0
#                  uses this so its chunks never wait on the dx8 DMA)
#  score_fp8: rope outputs quantized e4m3, DR broadcast trick (0.9e-2/pair)
PAIR_CFG = [("3p", False), ("2p", True), ("3p", True), ("3p", True)]
# processing order: cheap fp8 pairs first (startup filler is heavy), the
# bf16-score pair last (no filler left to dribble by then)
PAIR_ORDER = [1, 2, 3, 0]
WARMUP_MMS = 6
TILE_BUDGET_NS = 900.0   # PE dribble budget per exp tile beyond fp8 scores


def _build_nc(debug: bool = False):
    nc = bacc.Bacc("TRN2", target_bir_lowering=False, debug=debug)

    d_x8 = nc.dram_tensor("x8", [P, KC, S], F8, kind="ExternalInput")
    d_dx8 = nc.dram_tensor("dx8", [P, KC, S], F8, kind="ExternalInput")
    d_wq = nc.dram_tensor("wq8", [P, NPAIR, KC, 2, P], F8, kind="ExternalInput")
    d_wk = nc.dram_tensor("wk8", [P, NPAIR, KC, 2, P], F8, kind="ExternalInput")
    d_wv = nc.dram_tensor("wv8", [P, KC, 2, OD], F8, kind="ExternalInput")
    d_cs = nc.dram_tensor("cs", [P, NQC, 2, QCH], BF16, kind="ExternalInput")
    d_R = nc.dram_tensor("rmat", [P, P], BF16, kind="ExternalInput")
    d_cf = nc.dram_tensor("cf", [P, 16], F32, kind="ExternalInput")  # bq|bk|mask
    d_bv = nc.dram_tensor("bv", [1, OD], F32, kind="ExternalInput")
    d_out = nc.dram_tensor("out", [S, OD], BF16, kind="ExternalOutput")

    _uid = [0]

    def _nm(pfx):
        _uid[0] += 1
        return f"{pfx}_{_uid[0]}"

    with tile.TileContext(nc) as tc, ExitStack() as ctx:
        const = ctx.enter_context(tc.tile_pool(name="const", bufs=1))
        qkp = ctx.enter_context(tc.tile_pool(name="qkp", bufs=8))
        rt = ctx.enter_context(tc.tile_pool(name="rt", bufs=14))
        ep = ctx.enter_context(tc.tile_pool(name="ep", bufs=30))
        op = ctx.enter_context(tc.tile_pool(name="op", bufs=3))
        npo = ctx.enter_context(tc.tile_pool(name="npo", bufs=8))
        pp = ctx.enter_context(tc.tile_pool(name="pp", bufs=2, space="PSUM"))
        sp = ctx.enter_context(tc.tile_pool(name="sp", bufs=2, space="PSUM"))
        cv = ctx.enter_context(tc.tile_pool(name="cv", bufs=2, space="PSUM"))

        # ---- SBUF constants / inputs ----
        t_x8 = const.tile([P, KC, S], F8)
        t_dx8 = const.tile([P, KC, S], F8)
        t_wq = const.tile([P, NPAIR, KC, 2, P], F8)
        t_wk = const.tile([P, NPAIR, KC, 2, P], F8)
        t_wv = const.tile([P, KC, 2, OD], F8)
        t_cs = const.tile([P, NQC, 2, QCH], BF16)
        t_R = const.tile([P, P], BF16)
        t_cf = const.tile([P, 16], F32)
        t_bv1 = const.tile([1, OD], F32)
        t_bvb = const.tile([P, OD], F32)
        t_v = [const.tile([P, HL, 65], BF16, tag=f"v{vt}", name=f"vsb{vt}") for vt in range(NT)]

        # small consts on the gpsimd ring (Pool-side DGE), big streams on sync
        nc.gpsimd.dma_start(t_cf[:], d_cf[:])
        nc.gpsimd.dma_start(t_R[:], d_R[:])
        nc.gpsimd.dma_start(t_bv1[:], d_bv[:])
        nc.gpsimd.dma_start(t_cs[:, 1], d_cs[:, 1])
        first = PAIR_ORDER[0]
        nc.sync.dma_start(t_x8[:, :, 0:QCH], d_x8[:, :, 0:QCH])
        nc.sync.dma_start(t_wq[:, first], d_wq[:, first])
        nc.sync.dma_start(t_wk[:, first], d_wk[:, first])
        nc.sync.dma_start(t_cs[:, 0], d_cs[:, 0])
        nc.sync.dma_start(t_dx8[:, :, 0:QCH], d_dx8[:, :, 0:QCH])
        nc.sync.dma_start(t_x8[:, :, QCH:S], d_x8[:, :, QCH:S])
        nc.sync.dma_start(t_wv[:], d_wv[:])
        nc.sync.dma_start(t_dx8[:, :, QCH:S], d_dx8[:, :, QCH:S])
        for hp in PAIR_ORDER[1:]:
            nc.sync.dma_start(t_wq[:, hp], d_wq[:, hp])
            nc.sync.dma_start(t_wk[:, hp], d_wk[:, hp])

        nc.gpsimd.partition_broadcast(t_bvb, t_bv1)

        # exp table warm (no DMA dependency)
        t_z = rt.tile([1, 4], F32, tag="warm", bufs=1)
        nc.vector.memset(t_z[:], 0.0)
        t_wz = rt.tile([1, 4], F32, tag="warm2", bufs=1)
        nc.scalar.activation(t_wz, t_z, AF.Exp)

        # PE HAM/p-state warmup during the input DMA wait
        t_wu = rt.tile([P, QCH], BF16, tag="wu", bufs=1)
        nc.vector.memset(t_wu[:], 0.0)
        p_wu = pp.tile([P, QCH], F32, tag="pp")
        for _ in range(WARMUP_MMS):
            nc.tensor.matmul(p_wu[:], t_wu[:, 0:P], t_wu[:], start=True, stop=True)

        # ---- chunk builders: lists of (est_ns, closure) + finalize ----
        def make_qk_chunk(hp, which, dest_tile, t, rot_on_pe=False, stt_eng=None):
            t_w = t_wq if which == "q" else t_wk
            tsl = slice(t * QCH, (t + 1) * QCH)
            mode = PAIR_CFG[hp][0]
            state = {}

            def get_p():
                if "p" not in state:
                    state["p"] = pp.tile([P, QCH], F32, tag="pp", name=_nm("pq"))
                return state["p"]

            mms = []
            if mode == "3p":
                def mk3(prod, cp):
                    def f():
                        p = get_p()
                        wi = 1 if prod == 1 else 0
                        rhs_t = t_dx8 if prod == 2 else t_x8
                        nc.tensor.matmul(
                            p,
                            t_w[:, hp, 2 * cp : 2 * cp + 2, wi, :],
                            rhs_t[:, 2 * cp : 2 * cp + 2, tsl],
                            start=(prod == 0 and cp == 0),
                            stop=(prod == 2 and cp == CP - 1),
                            perf_mode=DR,
                        )
                    return f
                mms = [(107.0, mk3(prod, cp)) for prod in range(3) for cp in range(CP)]
            else:
                def mk2(c):
                    def f():
                        p = get_p()
                        nc.tensor.matmul(
                            p,
                            t_w[:, hp, c, :, :],
                            t_x8[:, c, tsl].unsqueeze(1).to_broadcast([P, 2, QCH]),
                            start=(c == 0),
                            stop=(c == KC - 1),
                            perf_mode=DR,
                        )
                    return f
                mms = [(107.0, mk2(c)) for c in range(KC)]

            def finalize():
                p = state["p"]
                eng = stt_eng or nc.vector
                scalar = t_cf[:, (0 if which == "q" else 4) + hp : (0 if which == "q" else 4) + hp + 1]
                t_ys = rt.tile([P, QCH], BF16, tag="ys", name=_nm("ys"))
                t_tc = rt.tile([P, QCH], BF16, tag="tc", name=_nm("tc"))
                eng.scalar_tensor_tensor(
                    out=t_ys, in0=p, scalar=scalar, in1=t_cs[:, t, 1], op0=ADD, op1=MUL
                )
                eng.scalar_tensor_tensor(
                    out=t_tc, in0=p, scalar=scalar, in1=t_cs[:, t, 0], op0=ADD, op1=MUL
                )
                if rot_on_pe:
                    # PE permutation matmul: shorter serial DVE chain — used
                    # for the startup-critical pre-stream chunks
                    nc.tensor.matmul(p, t_R[:], t_ys, start=True, stop=True)
                    nc.vector.tensor_add(dest_tile[:], t_tc, p)
                else:
                    # rotate-half as 4 DVE partition-block copies (bf16 SBUF,
                    # 4x mode) -- keeps the PE out of the rope chain; the add
                    # (all-SBUF) runs on the idle GpSimd to decongest DVE
                    t_rot = rt.tile([P, QCH], BF16, tag="rot", name=_nm("rot"))
                    for blk in (0, 64):
                        nc.vector.tensor_copy(t_rot[blk : blk + 32], t_ys[blk + 32 : blk + 64])
                        nc.vector.tensor_copy(t_rot[blk + 32 : blk + 64], t_ys[blk : blk + 32])
                    nc.gpsimd.tensor_add(dest_tile[:], t_tc, t_rot)

            mms.append((0.0, finalize))
            return mms

        def make_v_chunk(vt):
            vtsl = slice(vt * P, (vt + 1) * P)
            state = {}

            def get_p():
                if "p" not in state:
                    state["p"] = pp.tile([P, OD], F32, tag="pp", name=_nm("pv"))
                return state["p"]

            def mkv(prod, cp):
                def f():
                    p = get_p()
                    lhs_t = t_dx8 if prod == 2 else t_x8
                    wi = 1 if prod == 1 else 0
                    nc.tensor.matmul(
                        p,
                        lhs_t[:, 2 * cp : 2 * cp + 2, vtsl],
                        t_wv[:, 2 * cp : 2 * cp + 2, wi, :],
                        start=(prod == 0 and cp == 0),
                        stop=(prod == 2 and cp == CP - 1),
                        perf_mode=DR,
                    )
                return f

            mms = [(107.0, mkv(prod, cp)) for prod in range(3) for cp in range(CP)]

            def finalize():
                p = state["p"]
                nc.vector.tensor_copy(
                    t_v[vt][:, :, 0:64],
                    p[:].rearrange("p (h c) -> p h c", h=HL, c=64),
                )
                # scaled-ones denominator column: V_hat is W_SCALE*V, so the
                # denominator must also be W_SCALE*sum(E) for the ratio to
                # stay correct
                nc.vector.memset(t_v[vt][:, :, 64:65], W_SCALE)

            mms.append((0.0, finalize))
            return mms

        def qk_tiles(hp):
            dt_ = F8 if PAIR_CFG[hp][1] else BF16
            qr = [qkp.tile([P, QCH], dt_, tag=f"qr{t}", name=f"qr{t}_{hp}") for t in range(NQC)]
            kr = [qkp.tile([P, QCH], dt_, tag=f"kr{t}", name=f"kr{t}_{hp}") for t in range(NQC)]
            return qr, kr

        # ---- pre-stream: first pair's q-t0/k-t0 chunks, token-split in
        # 256-col halves for a shorter serial rope chain; q's stts on DVE,
        # k's on the idle GpSimd; rotates land in a borrowed scores-psum
        # tile (sequential groups per bank) ----
        p_first = PAIR_ORDER[0]
        pair_tiles = {p_first: qk_tiles(p_first)}
        qr0, kr0 = pair_tiles[p_first]
        HQ = QCH // 2
        p_preq = pp.tile([P, QCH], F32, tag="pp", name=_nm("ppreq"))
        p_prek = pp.tile([P, QCH], F32, tag="pp", name=_nm("pprek"))
        s_rot = sp.tile([P, 2 * QCH], F32, tag="sp", name=_nm("srot"))

        def pre_mms(which, p_c, half):
            t_w = t_wq if which == "q" else t_wk
            csl = slice(half * HQ, (half + 1) * HQ)
            for c in range(KC):
                nc.tensor.matmul(
                    p_c[:, csl],
                    t_w[:, p_first, c, :, :],
                    t_x8[:, c, csl].unsqueeze(1).to_broadcast([P, 2, HQ]),
                    start=(c == 0), stop=(c == KC - 1), perf_mode=DR,
                )

        def pre_fin(which, p_c, dest, half, rot_off, eng, add_eng=None):
            scalar = t_cf[:, (0 if which == "q" else 4) + p_first : (0 if which == "q" else 4) + p_first + 1]
            csl = slice(half * HQ, (half + 1) * HQ)
            t_ys = rt.tile([P, HQ], BF16, tag="ys", name=_nm("ys"))
            t_tc = rt.tile([P, HQ], BF16, tag="tc", name=_nm("tc"))
            eng.scalar_tensor_tensor(
                out=t_ys, in0=p_c[:, csl], scalar=scalar, in1=t_cs[:, 0, 1, csl], op0=ADD, op1=MUL
            )
            eng.scalar_tensor_tensor(
                out=t_tc, in0=p_c[:, csl], scalar=scalar, in1=t_cs[:, 0, 0, csl], op0=ADD, op1=MUL
            )
            nc.tensor.matmul(
                s_rot[:, rot_off : rot_off + HQ], t_R[:], t_ys, start=True, stop=True
            )
            (add_eng or nc.vector).tensor_add(
                dest[:, csl], t_tc, s_rot[:, rot_off : rot_off + HQ]
            )

        # q: one full-width chunk (3 DVE ops — fewer per-op overheads);
        # k: two halves so kr[:, 0:256] unblocks the first scores early
        for c in range(KC):
            nc.tensor.matmul(
                p_preq,
                t_wq[:, p_first, c, :, :],
                t_x8[:, c, 0:QCH].unsqueeze(1).to_broadcast([P, 2, QCH]),
                start=(c == 0), stop=(c == KC - 1), perf_mode=DR,
            )
        pre_mms("k", p_prek, 0)
        q_scalar = t_cf[:, p_first : p_first + 1]
        t_ysq = rt.tile([P, QCH], BF16, tag="ys", name=_nm("ys"))
        t_tcq = rt.tile([P, QCH], BF16, tag="tc", name=_nm("tc"))
        nc.vector.scalar_tensor_tensor(
            out=t_ysq, in0=p_preq, scalar=q_scalar, in1=t_cs[:, 0, 1], op0=ADD, op1=MUL
        )
        nc.vector.scalar_tensor_tensor(
            out=t_tcq, in0=p_preq, scalar=q_scalar, in1=t_cs[:, 0, 0], op0=ADD, op1=MUL
        )
        nc.tensor.matmul(s_rot[:, 0:QCH], t_R[:], t_ysq, start=True, stop=True)
        nc.vector.tensor_add(qr0[0][:], t_tcq, s_rot[:, 0:QCH])
        pre_mms("k", p_prek, 1)
        pre_fin("k", p_prek, kr0[0], 0, 2 * QCH - 2 * HQ, nc.vector)
        pre_fin("k", p_prek, kr0[0], 1, 2 * QCH - HQ, nc.vector)

        # ---- filler: remaining chunk work, dribbled into PE slack ----
        # items are (est_ns, closure, seg); need(seg) force-pops (in FIFO
        # order) until that segment is fully issued — issue deadlines.
        filler = deque()
        seg_left = {}

        def fpush(seg, items):
            seg_left[seg] = seg_left.get(seg, 0) + len(items)
            for est, f in items:
                filler.append((est, f, seg))

        fpush(f"p{p_first}k1", make_qk_chunk(p_first, "k", kr0[1], 1))
        fpush(f"p{p_first}q1", make_qk_chunk(p_first, "q", qr0[1], 1))
        for hp in PAIR_ORDER[1:]:
            pair_tiles[hp] = qk_tiles(hp)
        pA, pB, pC = PAIR_ORDER[1], PAIR_ORDER[2], PAIR_ORDER[3]

        def qk_seg(hp, which, t):
            qr, kr = pair_tiles[hp]
            dest = (qr if which == "q" else kr)[t]
            fpush(f"p{hp}{which}{t}", make_qk_chunk(hp, which, dest, t))

        qk_seg(pA, "q", 0)
        qk_seg(pA, "k", 0)
        fpush("v0", make_v_chunk(0))
        fpush("v1", make_v_chunk(1))
        qk_seg(pA, "k", 1)
        fpush("v2", make_v_chunk(2))
        fpush("v3", make_v_chunk(3))
        qk_seg(pA, "q", 1)
        fpush("v4", make_v_chunk(4))
        fpush("v5", make_v_chunk(5))
        fpush("v6", make_v_chunk(6))
        fpush("v7", make_v_chunk(7))
        qk_seg(pB, "q", 0)
        qk_seg(pB, "k", 0)
        qk_seg(pB, "k", 1)
        qk_seg(pB, "q", 1)
        # (pair C's chunks follow; all V is issued by here so early units'
        # PV groups unblock and es slots recycle)
        qk_seg(pC, "q", 0)
        qk_seg(pC, "k", 0)
        qk_seg(pC, "k", 1)
        qk_seg(pC, "q", 1)

        vt_issued = [0]  # count of fully-issued V chunks

        def pop_one():
            est, f, seg = filler.popleft()
            f()
            seg_left[seg] -= 1
            if est == 0.0 and seg.startswith("v"):
                vt_issued[0] += 1
            return est

        def pop_filler(budget_ns):
            spent = 0.0
            while filler and spent < budget_ns:
                spent += pop_one()

        def need(seg):
            while seg_left.get(seg, 0) > 0:
                pop_one()

        # pvq: FIFO of (min_vt, est_ns, closure) for PV accumulation groups
        # (one PSUM bank each: 8 MMs + reciprocal + normalize) and out-DMAs
        pvq = deque()

        def pop_pvq(budget_ns):
            spent = 0.0
            while pvq and pvq[0][0] <= vt_issued[0] and spent < budget_ns:
                _, est, f = pvq.popleft()
                f()
                spent += est
            return spent

        d_or = d_out[:].rearrange("(a j p) (hp c) -> p a j hp c", p=P, j=NJ, hp=NPAIR)

        def push_unit_groups(hp_, qc_, es_list, t_out_):
            for j in range(NJ):
                for h in range(2):
                    gi = 2 * j + h
                    last_u = hp_ == PAIR_ORDER[-1] and qc_ == NQC - 1

                    def g(j=j, h=h, gi=gi, es_list=es_list, t_out_=t_out_, hp_=hp_, last_u=last_u):
                        def f():
                            pool_ = pp if (last_u and gi % 2 == 1) else cv
                            p_g = pool_.tile([P, 65], F32, tag="cv" if pool_ is cv else "pp", name=_nm("cv"))
                            for kt in range(NT):
                                nc.tensor.matmul(
                                    p_g,
                                    es_list[kt][:, h * QCH + j * P : h * QCH + (j + 1) * P],
                                    t_v[kt][:, 2 * hp_ + h, :],
                                    start=(kt == 0), stop=(kt == NT - 1),
                                )
                            t_rr = npo.tile([P, 1], F32, tag="rr", name=_nm("rr"))
                            nc.vector.reciprocal(t_rr, p_g[:, 64:65])
                            nc.vector.scalar_tensor_tensor(
                                out=t_out_[:, j, h * DH : (h + 1) * DH],
                                in0=p_g[:, 0:DH],
                                scalar=t_rr,
                                in1=t_bvb[:, (2 * hp_ + h) * DH : (2 * hp_ + h + 1) * DH],
                                op0=MUL, op1=ADD,
                            )
                        return f
                    pvq.append((NT, 250.0, g()))

            last_unit = hp_ == PAIR_ORDER[-1] and qc_ == NQC - 1
            if last_unit:
                # per-j-pair DMAs, re-ordered into the group list so each
                # fires as soon as its half of t_out is normalized
                items = list(pvq)
                for _ in range(8):
                    pvq.pop()
                gs = items[-8:]
                del items
                for j0 in range(0, NJ, 2):
                    def dmaj(qc_=qc_, hp_=hp_, t_out_=t_out_, j0=j0):
                        nc.sync.dma_start(
                            d_or[:, qc_, j0 : j0 + 2, hp_, :], t_out_[:, j0 : j0 + 2]
                        )
                    pvq.extend(gs[4 * (j0 // 2) : 4 * (j0 // 2) + 4])
                    pvq.append((NT, 0.0, dmaj))
            else:
                def dma(qc_=qc_, hp_=hp_, t_out_=t_out_):
                    nc.sync.dma_start(d_or[:, qc_, :, hp_, :], t_out_[:])
                pvq.append((NT, 0.0, dma))

        # ---- the ACT-paced attention stream ----
        def pv_mm(p_g, es_t, j, h, hp_, kt):
            nc.tensor.matmul(
                p_g,
                es_t[:, h * QCH + j * P : h * QCH + (j + 1) * P],
                t_v[kt][:, 2 * hp_ + h, :],
                start=(kt == 0), stop=(kt == NT - 1),
            )

        def norm_group(p_g, j, h, hp_, t_out_, stt_eng):
            t_rr = npo.tile([P, 1], F32, tag="rr", name=_nm("rr"))
            nc.vector.reciprocal(t_rr, p_g[:, 64:65])
            stt_eng.scalar_tensor_tensor(
                out=t_out_[:, j, h * DH : (h + 1) * DH],
                in0=p_g[:, 0:DH],
                scalar=t_rr,
                in1=t_bvb[:, (2 * hp_ + h) * DH : (2 * hp_ + h + 1) * DH],
                op0=MUL, op1=ADD,
            )

        unit_no = [0]
        for hp in PAIR_ORDER:
            fp8_pair = PAIR_CFG[hp][1]
            qr_cur, kr_cur = pair_tiles[hp]
            for qc in range(NQC):
                last_unit = hp == PAIR_ORDER[-1] and qc == NQC - 1
                need(f"p{hp}q{qc}")
                if qc == 0:
                    need(f"p{hp}k0")
                t_out = op.tile([P, NJ, 2 * DH], BF16, tag="out")
                streamed = []
                es_tiles = []
                for kt in range(NT):
                    if kt == 4:
                        need(f"p{hp}k1")
                    # scores for this k-tile (2 heads)
                    p_s = sp.tile([P, 2 * QCH], F32, tag="sp")
                    krc = kr_cur[kt // 4]
                    kcol = (kt % 4) * P
                    for half, b0 in ((0, 0), (1, 64)):
                        osl = slice(half * QCH, (half + 1) * QCH)
                        if fp8_pair:
                            nc.tensor.matmul(
                                p_s[:, osl],
                                krc[b0 : b0 + 64, kcol : kcol + P]
                                .unsqueeze(1).to_broadcast([64, 2, P]),
                                qr_cur[qc][b0 : b0 + 64, :]
                                .unsqueeze(1).to_broadcast([64, 2, QCH]),
                                start=True, stop=True, perf_mode=DR,
                            )
                        else:
                            nc.tensor.matmul(
                                p_s[:, osl],
                                krc[b0 : b0 + 64, kcol : kcol + P],
                                qr_cur[qc][b0 : b0 + 64, :],
                                start=True, stop=True,
                            )
                    t_e = ep.tile([P, 2 * QCH], BF16, tag="e")
                    nc.scalar.activation(
                        t_e, p_s, AF.Exp, bias=t_cf[:, 8 + kt : 9 + kt],
                        scale=0.0625 if fp8_pair else 0.125,
                    )
                    es_tiles.append(t_e)

                    s_ns = 213.0 if fp8_pair else 854.0
                    gb = 900.0 if not filler else (500.0 if len(pvq) > 18 else 300.0)
                    g_spent = pop_pvq(gb)
                    fb = TILE_BUDGET_NS + (500.0 if vt_issued[0] >= NT else 0.0) \
                        - (150.0 if unit_no[0] == 0 else 0.0)
                    pop_filler(max(0.0, fb - (s_ns - 213.0) - g_spent))

                push_unit_groups(hp, qc, es_tiles, t_out)
                unit_no[0] += 1

        # drain any remaining queued work (last unit's PV + tails)
        while pvq or filler:
            pop_filler(1e9)
            pop_pvq(1e9)

    nc.compile()
    return nc


_NC_CACHE = {}


def _get_nc(debug: bool = False):
    key = bool(debug)
    if key not in _NC_CACHE:
        _NC_CACHE[key] = _build_nc(debug)
    return _NC_CACHE[key]


def _prep_inputs(hidden_states, attention_mask, freqs, Wq, bq, Wk, bk, Wv, bv):
    hidden_states = np.asarray(hidden_states)
    attention_mask = np.asarray(attention_mask)
    freqs = np.asarray(freqs)
    Wq, bq = np.asarray(Wq), np.asarray(bq)
    Wk, bk = np.asarray(Wk), np.asarray(bk)
    Wv, bv = np.asarray(Wv), np.asarray(bv)
    bf = ml_dtypes.bfloat16
    f8 = mybir.dt.np(F8)

    inv = 1.0 / W_SCALE
    cosf = (np.cos(freqs.astype(np.float64)) * inv).astype(np.float32)  # [S, 64]
    sinf = (np.sin(freqs.astype(np.float64)) * inv).astype(np.float32)
    # sign-baked sin for the rotate-half permutation matmul
    sgn = np.ones((64, 1), np.float32)
    sgn[32:] = -1.0
    cos2 = np.tile(cosf.T, (2, 1))                    # [128, S]
    sin2 = np.tile(sinf.T * sgn, (2, 1))              # [128, S], sign-baked
    # cs layout [P, NQC, 2, QCH]
    cs = np.empty((P, NQC, 2, QCH), np.float32)
    for t in range(NQC):
        cs[:, t, 0] = cos2[:, t * QCH : (t + 1) * QCH]
        cs[:, t, 1] = sin2[:, t * QCH : (t + 1) * QCH]
    cs = cs.astype(bf)

    rmat = np.zeros((P, P), np.float32)
    for blk in (0, 64):
        for i in range(32):
            rmat[blk + i + 32, blk + i] = 1.0
            rmat[blk + i, blk + i + 32] = 1.0
    rmat = rmat.astype(bf)

    def split8(a):
        a8 = a.astype(f8)
        d8 = (a - a8.astype(np.float32)).astype(f8)
        return a8, d8

    # x transposed [HID, S] -> [P, KC, S]
    x8s, dx8s = [], []
    for b in range(B):
        xt = np.ascontiguousarray(hidden_states[b].T).astype(np.float32)
        x8, dx8 = split8(xt)
        x8s.append(np.ascontiguousarray(x8.reshape(KC, P, S).transpose(1, 0, 2)))
        dx8s.append(np.ascontiguousarray(dx8.reshape(KC, P, S).transpose(1, 0, 2)))

    masks = []
    for b in range(B):
        m = np.where(attention_mask[b, 0, 0, :] < -1e-5, MASK_NEG, 0.0).astype(np.float32)
        masks.append(np.ascontiguousarray(m.reshape(NT, P).T))

    def pack_w_qk(Wm):
        # [HID, OD] -> [P, NPAIR, KC, 2, P] with (w8, dw8) in axis 3
        w = (Wm * W_SCALE).astype(np.float32)
        w8, dw8 = split8(w)
        out = np.empty((P, NPAIR, KC, 2, P), np.float32)
        for hp in range(NPAIR):
            csl = slice(hp * P, (hp + 1) * P)
            out[:, hp, :, 0, :] = w8.astype(np.float32)[:, csl].reshape(KC, P, P).transpose(1, 0, 2)
            out[:, hp, :, 1, :] = dw8.astype(np.float32)[:, csl].reshape(KC, P, P).transpose(1, 0, 2)
        return np.ascontiguousarray(out.astype(f8))

    def pack_w_v(Wm):
        w = (Wm * W_SCALE).astype(np.float32)
        w8, dw8 = split8(w)
        out = np.empty((P, KC, 2, OD), np.float32)
        out[:, :, 0, :] = w8.astype(np.float32).reshape(KC, P, OD).transpose(1, 0, 2)
        out[:, :, 1, :] = dw8.astype(np.float32).reshape(KC, P, OD).transpose(1, 0, 2)
        return np.ascontiguousarray(out.astype(f8))

    wqs, wks, wvs, cfs, bvs = [], [], [], [], []
    for g in range(HG):
        osl = slice(g * OD, (g + 1) * OD)
        wqs.append(pack_w_qk(Wq[:, osl]))
        wks.append(pack_w_qk(Wk[:, osl]))
        wvs.append(pack_w_v(Wv[:, osl]))
        bqs = (bq[osl] * W_SCALE).reshape(NPAIR, P).T       # [P, 4]
        bks = (bk[osl] * W_SCALE).reshape(NPAIR, P).T
        cfs.append((bqs, bks))
        bvs.append(bv[osl].reshape(1, OD).astype(np.float32))

    in_maps = []
    for c in range(NCORES):
        b, g = c // HG, c % HG
        cf = np.concatenate([cfs[g][0], cfs[g][1], masks[b]], axis=1).astype(np.float32)
        assert cf.shape == (P, 16)
        in_maps.append(
            dict(
                x8=x8s[b], dx8=dx8s[b],
                wq8=wqs[g], wk8=wks[g], wv8=wvs[g],
                cs=cs, rmat=rmat, cf=cf, bv=bvs[g],
            )
        )
    return in_maps


def kernel(hidden_states, attention_mask, freqs, Wq, bq, Wk, bk, Wv, bv, **run_kwargs):
    nc = _get_nc()
    in_maps = _prep_inputs(
        hidden_states, attention_mask, freqs, Wq, bq, Wk, bk, Wv, bv
    )
    res = run_bass_kernel_spmd(nc, in_maps, core_ids=list(range(NCORES)), **run_kwargs)
    out = np.empty((B, S, H * DH), np.float32)
    for c in range(NCORES):
        b, g = c // HG, c % HG
        out[b, :, g * OD : (g + 1) * OD] = np.asarray(res.results[c]["out"]).astype(np.float32)
    if run_kwargs:
        kernel.last_results = res
    return out
